# revision 30
# baseline (speedup 1.0000x reference)
"""Trainium2 Bass kernel for nn_MLA_KVSplice (MLA attention with KVSplice
latent bottleneck), tensor-parallel over heads across 8 NeuronCores.

v2: the whole latent pipeline is folded on the host.  kv_latent is only
consumed by the splice, and LN is a per-token affine, so:

  y^T   = Wfold @ x^T + cvec        Wfold = Wc.diag(softplus(t_scale)).Wkl
  K^T   = (W2 @ y^T).At - s2 (mu.At)^T + bk_eff     W2 = WfK_c @ We_g
  V     = ((W3 @ y^T)^T).At_col - (mu.At) s3^T + bv_eff

so the on-device contraction chain is x[2048] -> y[256] -> K/V[256]: the
512-wide latent matmul, the compress input and the expand matmul all
disappear (~48us of PE per core vs v1).

Per core c (heads {2c, 2c+1}):
  - All big tensors live/compute in transposed layouts so every matmul
    contraction sits on the partition dim; DRAM inputs are pre-laid
    host-side in exact SBUF layout.
  - Startup: wfold + x chunk0 stream in ramped pieces on the sync+scalar
    queues; wq follows so chunk-0 Q can start ko-wise as it lands.
  - LN stats per chunk via ones-matmul (row broadcast form); column forms
    (for V) extracted from partition 0 with tiny transposing DMAs.
  - K/V of chunk i-1 are emitted inside chunk i so the PE never stalls on
    the stats DVE chain.
  - Per-head causal attention in S^T[j,i] layout: exp without
    max-subtraction, row-sums via ones-matmul, fully masked j-tiles
    skipped.
  - Row-parallel out-proj staggered into the attention loop; each core
    emits a fp16 partial [T, D]; host sums the 8 partials in fp32 + bout.
"""

import math
import os

import numpy as np

import concourse.bass as bass
import concourse.tile as tile
from concourse import bacc, mybir
from concourse.bass_utils import run_bass_kernel_spmd

# problem constants (hardcoded per harness contract)
B, T, D = 1, 2048, 2048
H, HD = 16, 128
DLAT, DCMP = 512, 256
THETA = 10000.0
LN_EPS = 1e-5
N_CORES = 8
HPC = H // N_CORES          # heads per core = 2
M = HPC * HD                # per-core head dims = 256

P = 128                     # partitions
TCH = 512                   # t-chunk for pass 1
NT = T // TCH               # 4
NKO = D // P                # 16 contraction chunks over model dim
NCC = DCMP // P             # 2
NIC = T // 512              # 4 i-chunks in attention
NJC = T // P                # 16 j-chunks
NTC16 = T // P              # 16 row-chunks in out-proj
NTS = TCH // P              # 4 t-subchunks per chunk

F16 = mybir.dt.float16
F32 = mybir.dt.float32
AF = mybir.ActivationFunctionType
ALU = mybir.AluOpType

ATT_SCALE = 1.0 / math.sqrt(HD)

_CACHE = {}

LAST_RESULT = None  # BassKernelResults of the most recent run (for test.py)


def _recip(nc, out, in_):
    """1/in_ on DVE; fast approx when available (18 bits, plenty here)."""
    if hasattr(nc.vector, "reciprocal_approx_fast"):
        nc.vector.reciprocal_approx_fast(out=out[:], in_=in_[:])
    else:
        nc.vector.reciprocal(out[:], in_[:])


def _build():
    if "nc" in _CACHE:
        return _CACHE["nc"]

    nc = bacc.Bacc(None, target_bir_lowering=False)

    def din(name, shape, dt):
        return nc.dram_tensor(name, shape, dt, kind="ExternalInput")

    # every input is pre-laid host-side in its exact SBUF layout
    xTp_d = din("xTp", [NT, P, NKO * TCH], F16)
    wfoldp_d = din("wfoldp", [P, NKO * DCMP], F16)
    wqp_d = din("wqp", [P, NKO, M], F16)
    w2p_d = din("w2p", [P, NCC, M], F16)
    w3p_d = din("w3p", [P, NCC, M], F16)
    woutp_d = din("woutp", [P, HPC, D], F16)
    cosp_d = din("cosp", [P, T], F16)
    sinp_d = din("sinp", [P, T], F16)
    maskp_d = din("maskp", [P, 4, 512], F16)
    onesp_d = din("onesp", [P, P], F16)
    # all small per-partition vectors in ONE tensor -> one DMA
    # cols: 0:2 cvec | 2:4 bq | 4:6 bk_eff | 6:8 neg_s2 | 8 eps
    vecs_d = din("vecs", [P, 9], F32)
    nbvp_d = din("nbvp", [P, 2, M], F16)   # [0]=neg_s3 bcast, [1]=bv_eff bcast

    out_d = nc.dram_tensor("out_partial", [NTC16, P, D], F16,
                           kind="ExternalOutput")

    with tile.TileContext(nc) as tc:
        with (
            tc.tile_pool(name="consts", bufs=1) as cp,
            tc.tile_pool(name="persist", bufs=1) as pp,
            tc.tile_pool(name="work", bufs=2) as wp,
            tc.tile_pool(name="psum", bufs=6, space="PSUM") as psp,
            tc.tile_pool(name="dscr", bufs=2, space="DRAM") as dp,
        ):
            # ---- tiny consts first on the gpsimd queue; the bulkier consts
            # gated behind most of the x0 stream by a blocker copy ----
            vecs = cp.tile([P, 9], F32, tag="vecs", name="vecs")
            nc.gpsimd.dma_start(vecs[:], vecs_d[:, :])
            ones16 = cp.tile([P, P], F16, tag="ones16", name="ones16")
            nc.gpsimd.dma_start(ones16[:], onesp_d[:, :])

            cvec = vecs[:, 0:NCC]
            bq2 = vecs[:, 2:2 + HPC]
            bk2 = vecs[:, 4:4 + HPC]
            ns2 = vecs[:, 6:6 + HPC]
            epsc = vecs[:, 8:9]

            # ---- all large ordered loads ride the SYNC queue alone: strict
            # arrival order, no competition for the shared DMA engines, and
            # the ACT queue stays free for copybacks.  x0+wfold stream in
            # ramped ko-pieces so the first matmul starts ~1us in. ----
            PIECES = [1, 1, 2, 4, 8]    # ko's per piece (ramped)
            wfoldT = cp.tile([P, NKO, DCMP], F16, tag="wfoldT",
                             name="wfoldT")
            xT_t0 = wp.tile([P, NKO, TCH], F16, tag="xT", name="xT_t0",
                            bufs=4)
            wf_r = wfoldp_d[:, :].rearrange("p (ko c) -> p ko c", c=DCMP)
            x0_r = xTp_d[0, :, :].rearrange("p (ko t) -> p ko t", t=TCH)
            # chunk 0 runs yT only (Q(0) is deferred into chunk 1), so
            # the critical startup stream is just x0 + wfold
            k0 = 0
            for kp in PIECES:
                ks = slice(k0, k0 + kp)
                nc.sync.dma_start(wfoldT[:, ks, :], wf_r[:, ks, :])
                nc.scalar.dma_start(xT_t0[:, ks, :], x0_r[:, ks, :])
                k0 += kp
            wqT = cp.tile([P, NKO, M], F16, tag="wqT", name="wqT")
            nc.scalar.dma_start(wqT[:, 0:NKO // 2, :],
                                wqp_d[:, 0:NKO // 2, :])
            nc.scalar.dma_start(wqT[:, NKO // 2:, :],
                                wqp_d[:, NKO // 2:, :])
            xT_t1 = wp.tile([P, NKO, TCH], F16, tag="xT", name="xT_t1",
                            bufs=4)
            nc.sync.dma_start(xT_t1[:],
                              xTp_d[1, :, :].rearrange("p (ko t) -> p ko t",
                                                       t=TCH))
            cosT = cp.tile([P, T], F16, tag="cosT", name="cosT")
            nc.scalar.dma_start(cosT[:], cosp_d[:, :])
            sinT = cp.tile([P, T], F16, tag="sinT", name="sinT")
            nc.scalar.dma_start(sinT[:], sinp_d[:, :])

            # K/V weights + mask on gpsimd, behind most of the x0 stream
            blk = cp.tile([P, 1], F16, tag="blk", name="blk")
            nc.gpsimd.tensor_copy(blk[:], xT_t0[:, sum(PIECES[:3]) - 1, 0:1])
            w2T = cp.tile([P, NCC, M], F16, tag="w2T", name="w2T")
            nc.gpsimd.dma_start(w2T[:], w2p_d[:, :, :])
            w3T = cp.tile([P, NCC, M], F16, tag="w3T", name="w3T")
            nc.gpsimd.dma_start(w3T[:], w3p_d[:, :, :])
            nbv = cp.tile([P, 2, M], F16, tag="nbv", name="nbv")
            nc.gpsimd.dma_start(nbv[:], nbvp_d[:, :, :])
            maskT = cp.tile([P, 4, 512], F16, tag="maskT", name="maskT")
            nc.gpsimd.dma_start(maskT[:], maskp_d[:, :, :])
            ns3_bc = nbv[:, 0, :]
            bv_bc = nbv[:, 1, :]

            # ---- persistent intermediates ----
            kT_all = pp.tile([P, HPC, T], F16, tag="kT_all", name="kT_all")
            qT_all = pp.tile([P, HPC, T], F16, tag="qT_all", name="qT_all")
            v_all = pp.tile([P, NJC, M], F16, tag="v_all", name="v_all")
            outT_all = pp.tile([P, HPC, T], F16, tag="outT_all",
                               name="outT_all")

            # ======== pass 1: y, stats, K/V, Q (sw-pipelined) ======
            prev = {}

            HHD = HD // 2
            rope_jobs = []

            def rope_issue(pre, dst_slice, tsl):
                """Half-rotation as an SBUF->SBUF partition swap on the idle
                DMA engines (rotation sign folded into sinT host-side).  The
                DVE combine is deferred to the chunk tail."""
                perm = wp.tile([P, TCH], F16, tag="perm", name="perm", bufs=12)
                nc.gpsimd.dma_start(perm[0:HHD, :], pre[HHD:P, :])
                nc.gpsimd.dma_start(perm[HHD:P, :], pre[0:HHD, :])
                rope_jobs.append((pre, perm, dst_slice, tsl))

            def rope_flush():
                for pre, perm, dst_slice, tsl in rope_jobs:
                    t1 = wp.tile([P, TCH], F16, tag="ropet1", name="rope_t1")
                    t2 = wp.tile([P, TCH], F16, tag="ropet2", name="rope_t2")
                    nc.vector.tensor_mul(t1[:], pre[:], cosT[:, tsl])
                    nc.vector.tensor_mul(t2[:], perm[:], sinT[:, tsl])
                    nc.vector.tensor_add(dst_slice, t1[:], t2[:])
                rope_jobs.clear()

            def emit_K(pv):
                """K^T[m, t] for chunk pv: (W2@yT).At - s2.Bt^T + bk."""
                tsl = pv["tsl"]
                for h in range(HPC):
                    psk = psp.tile([P, TCH], F32, tag="ps", name="ps_k")
                    for cc in range(NCC):
                        nc.tensor.matmul(
                            psk[:],
                            w2T[:, cc, h * HD:(h + 1) * HD],
                            pv["yT"][:, cc, :],
                            start=(cc == 0), stop=(cc == NCC - 1),
                        )
                    tmp = wp.tile([P, TCH], F16, tag="ktmp", name="ktmp")
                    nc.vector.tensor_mul(tmp[:], psk[:], pv["At"][:])
                    kst = wp.tile([P, TCH], F16, tag="kst", name="kst")
                    nc.vector.scalar_tensor_tensor(
                        kst[:], pv["Bt"][:], ns2[:, h:h + 1], tmp[:],
                        op0=ALU.mult, op1=ALU.add,
                    )
                    kpre = wp.tile([P, TCH], F16, tag="kpre", name="kpre")
                    nc.scalar.activation(kpre[:], kst[:], AF.Identity,
                                         bias=bk2[:, h:h + 1])
                    rope_issue(kpre, kT_all[:, h, tsl], tsl)

            def emit_V(pv):
                """V[t, m] for chunk pv: (W3@yT)^T.At_col - Bt_col.s3 + bv."""
                tci = pv["tci"]
                for ts4 in range(NTS):
                    psv = psp.tile([P, TCH], F32, tag="ps", name="ps_v")
                    for cc in range(NCC):
                        nc.tensor.matmul(
                            psv[:, 0:M],
                            pv["yT"][:, cc, ts4 * P:(ts4 + 1) * P],
                            w3T[:, cc, :],
                            start=(cc == 0), stop=(cc == NCC - 1),
                        )
                    v1 = wp.tile([P, M], F16, tag="v1", name="v1")
                    nc.scalar.mul(v1[:], psv[:, 0:M],
                                  pv["Atc"][:, ts4:ts4 + 1])
                    v2 = wp.tile([P, M], F16, tag="v2", name="v2")
                    nc.vector.scalar_tensor_tensor(
                        v2[:], ns3_bc, pv["Btc"][:, ts4:ts4 + 1], v1[:],
                        op0=ALU.mult, op1=ALU.add,
                    )
                    nc.vector.tensor_add(v_all[:, tci * NTS + ts4, :],
                                         v2[:], bv_bc)

            def emit_Q(xT_t, tsl, psq=None):
                """Both heads; psq supplies precomputed chunk-0 psums."""
                if psq is None:
                    psq = [psp.tile([P, TCH], F32, tag="ps",
                                    name=f"ps_q{hc}")
                           for hc in range(HPC)]
                    for hc in range(HPC):
                        for ko in range(NKO):
                            nc.tensor.matmul(
                                psq[hc][:],
                                wqT[:, ko, hc * HD:(hc + 1) * HD],
                                xT_t[:, ko, :],
                                start=(ko == 0), stop=(ko == NKO - 1),
                            )
                for hc in range(HPC):
                    qpre = wp.tile([P, TCH], F16, tag="qpre", name="qpre",
                                   bufs=4)
                    nc.scalar.activation(qpre[:], psq[hc][:], AF.Identity,
                                         bias=bq2[:, hc:hc + 1])
                    rope_issue(qpre, qT_all[:, hc, tsl], tsl)

            for tci in range(NT):
                tsl = slice(tci * TCH, (tci + 1) * TCH)
                if tci == 0:
                    xT_t = xT_t0
                elif tci == 1:
                    xT_t = xT_t1
                else:
                    xT_t = wp.tile([P, NKO, TCH], F16, tag="xT", name="xT_t",
                                   bufs=4)
                    xi_r = xTp_d[tci, :, :].rearrange("p (ko t) -> p ko t",
                                                      t=TCH)
                    nc.sync.dma_start(xT_t[:], xi_r[:, :, :])
                    if tci == 2:
                        # out-proj weights (pass 2/3 only) on the scalar
                        # queue so x3 isn't delayed behind them
                        woutT = cp.tile([P, HPC, D], F16, tag="woutT",
                                        name="woutT")
                        nc.scalar.dma_start(woutT[:], woutp_d[:, :, :])

                # --- y^T = Wfold @ x^T + cvec  [DCMP, t] ---
                yT_t = wp.tile([P, NCC, TCH], F16, tag="yT", name="yT_t",
                               bufs=2)
                ysq_t = wp.tile([P, NCC, TCH], F16, tag="ysq", name="ysq_t",
                                bufs=1)
                if tci == 0:
                    # ko-outer, Q interleaved: consume each startup x piece
                    # once for both yT and Q as the pieces land
                    ps2c = [psp.tile([P, TCH], F32, tag="ps",
                                     name=f"ps_y{cc}")
                            for cc in range(NCC)]
                    for ko in range(NKO):
                        for cc in range(NCC):
                            nc.tensor.matmul(
                                ps2c[cc][:],
                                wfoldT[:, ko, cc * P:(cc + 1) * P],
                                xT_t[:, ko, :],
                                start=(ko == 0), stop=(ko == NKO - 1),
                                skip_group_check=True,
                            )
                    for cc in range(NCC):
                        nc.scalar.activation(yT_t[:, cc], ps2c[cc][:],
                                             AF.Identity,
                                             bias=cvec[:, cc:cc + 1])
                        nc.scalar.square(ysq_t[:, cc], yT_t[:, cc])
                else:
                    for cc in range(NCC):
                        psy = psp.tile([P, TCH], F32, tag="ps", name="ps_y")
                        for ko in range(NKO):
                            nc.tensor.matmul(
                                psy[:],
                                wfoldT[:, ko, cc * P:(cc + 1) * P],
                                xT_t[:, ko, :],
                                start=(ko == 0), stop=(ko == NKO - 1),
                            )
                        nc.scalar.activation(yT_t[:, cc], psy[:],
                                             AF.Identity,
                                             bias=cvec[:, cc:cc + 1])
                        nc.scalar.square(ysq_t[:, cc], yT_t[:, cc])

                # chunk i-1's K: fills the yT copyback latency before the
                # stats matmuls need it
                if prev:
                    emit_K(prev)

                # --- LN stats over DCMP via ones-matmul (bcast to 128p) ---
                ps1 = psp.tile([P, TCH], F32, tag="acA", name="ps_s1", bufs=1)
                ps2 = psp.tile([P, TCH], F32, tag="acB", name="ps_s2", bufs=1)
                for cc in range(NCC):
                    nc.tensor.matmul(ps1[:], ones16[:], yT_t[:, cc],
                                     start=(cc == 0), stop=(cc == NCC - 1),
                                     skip_group_check=True)
                for cc in range(NCC):
                    nc.tensor.matmul(ps2[:], ones16[:], ysq_t[:, cc],
                                     start=(cc == 0), stop=(cc == NCC - 1),
                                     skip_group_check=True)

                # chunk i-1's V (fills the stats copyback latency)
                if prev:
                    emit_V(prev)
                    if tci == NT - 1:
                        # flush K(i-1) ropes early: att(2) needs kT of
                        # chunk 2 before this chunk's Q ropes
                        rope_flush()

                mu = wp.tile([P, TCH], F32, tag="mu", name="mu", bufs=1)
                musq = wp.tile([P, TCH], F32, tag="musq", name="musq", bufs=1)
                m2 = wp.tile([P, TCH], F32, tag="m2", name="m2", bufs=1)
                std = wp.tile([P, TCH], F32, tag="std", name="std", bufs=1)
                At = wp.tile([P, TCH], F32, tag="At", name="At", bufs=2)
                Bt = wp.tile([P, TCH], F32, tag="Bt", name="Bt", bufs=2)
                nc.scalar.mul(mu[:], ps1[:], 1.0 / DCMP)
                # m2 = E[y^2] + eps  (eps folded into the copyback bias)
                nc.scalar.activation(m2[:], ps2[:], AF.Identity,
                                     bias=epsc[:, 0:1], scale=1.0 / DCMP)
                nc.scalar.square(musq[:], mu[:])
                nc.vector.tensor_sub(m2[:], m2[:], musq[:])
                nc.scalar.sqrt(std[:], m2[:])
                _recip(nc, At, std)
                nc.vector.tensor_mul(Bt[:], mu[:], At[:])
                # column forms for V: partition-0 row -> DRAM -> transposed
                # read-back (SBUF->SBUF DMAs cannot cross partitions)
                cst = dp.tile([2, TCH], F32, tag="cst", name="cst", bufs=2)
                nc.gpsimd.dma_start(cst[0:1, :], At[0:1, :])
                nc.gpsimd.dma_start(cst[1:2, :], Bt[0:1, :])
                Atc = wp.tile([P, NTS], F32, tag="Atc", name="Atc", bufs=2)
                Btc = wp.tile([P, NTS], F32, tag="Btc", name="Btc", bufs=2)
                nc.gpsimd.dma_start(
                    Atc[:], cst[0, :].rearrange("(c p) -> p c", p=P))
                nc.gpsimd.dma_start(
                    Btc[:], cst[1, :].rearrange("(c p) -> p c", p=P))

                # --- Q (+bias): independent filler over the LN chain;
                # chunk-0's Q runs here in chunk 1 (startup is DMA-bound) ---
                if tci > 0:
                    emit_Q(xT_t, tsl)
                if tci == 1:
                    emit_Q(xT_t0, slice(0, TCH))

                # rope combines for K(i-1) and Q(i); chunk-0's defer to
                # chunk 1 so cos/sin can load late
                if tci > 0:
                    rope_flush()

                prev = {"tci": tci, "tsl": tsl, "yT": yT_t,
                        "At": At, "Bt": Bt, "Atc": Atc, "Btc": Btc}

            # ====== pass 2 + staggered pass 3 (attention + out-proj) ======
            def outproj_unit(tt, dc, sbd, tail=False):
                """One (row-chunk, col-chunk) unit of the out projection."""
                if dc == 0:
                    sbd[tt] = wp.tile([P, D // 512, 512], F16, tag="outsb",
                                      name="out_sb", bufs=3)
                out_sb = sbd[tt]
                psp_o = psp.tile([P, 512], F32, tag="ps", name="ps_P")
                for hc in range(HPC):
                    nc.tensor.matmul(
                        psp_o[:],
                        outT_all[:, hc, tt * P:(tt + 1) * P],
                        woutT[:, hc, dc * 512:(dc + 1) * 512],
                        start=(hc == 0), stop=(hc == HPC - 1),
                    )
                # ACT is exp-loaded mid-kernel: all psum copies on DVE
                use_act = False if not tail else (dc % 2 == 0)
                if use_act:
                    nc.scalar.copy(out_sb[:, dc], psp_o[:])
                else:
                    nc.vector.tensor_copy(out_sb[:, dc], psp_o[:])
                if dc == D // 512 - 1:
                    nc.sync.dma_start(out_d[tt, :, :],
                                      out_sb[:].rearrange("p a b -> p (a b)"))

            def outproj_units(ic, tail=False):
                sbd = {}
                return [
                    (lambda tt=tt, dc=dc: outproj_unit(tt, dc, sbd, tail))
                    for tt in range(4 * ic, 4 * ic + 4)
                    for dc in range(D // 512)
                ]

            def emit_outproj(ic, tail=False):
                for u in outproj_units(ic, tail):
                    u()

            def emit_att(ic, fillers=(), fill_every=1):
                fillers = list(fillers)
                isl = slice(ic * 512, (ic + 1) * 512)
                njc = 4 * ic + 4
                for h in range(HPC):
                    ps_l = psp.tile([P, 512], F32, tag="acA", name="ps_L",
                                    bufs=1)
                    ps_o = psp.tile([P, 512], F32, tag="acB", name="ps_O",
                                    bufs=1)
                    WIN = 4
                    Pts = {}

                    def lv(jc, ps_l=ps_l, ps_o=ps_o, njc=njc, h=h, Pts=Pts):
                        Pt, c0 = Pts.pop(jc)
                        nc.tensor.matmul(ps_l[:, c0:], ones16[:], Pt[:, c0:],
                                         start=(jc == 0), stop=(jc == njc - 1),
                                         skip_group_check=True)
                        nc.tensor.matmul(ps_o[:, c0:],
                                         v_all[:, jc, h * HD:(h + 1) * HD],
                                         Pt[:, c0:],
                                         start=(jc == 0), stop=(jc == njc - 1),
                                         skip_group_check=True)

                    for jc in range(njc):
                        dd = jc - 4 * ic
                        # diagonal blocks: columns i < 128*dd are fully
                        # masked -> skip them in S/exp/L/O
                        c0 = P * dd if dd > 0 else 0
                        ps_s = psp.tile([P, 512], F32, tag="ps", name="ps_S")
                        nc.tensor.matmul(
                            ps_s[:, c0:],
                            kT_all[:, h, jc * P:(jc + 1) * P],
                            qT_all[:, h, ic * 512 + c0:(ic + 1) * 512],
                            start=True, stop=True,
                        )
                        Pt = wp.tile([P, 512], F16, tag="P", name="P_t",
                                     bufs=6)
                        nc.scalar.activation(Pt[:, c0:], ps_s[:, c0:], AF.Exp,
                                             scale=ATT_SCALE)
                        if dd >= 0:
                            # causal zeroing post-exp as a cheap fp16 SBUF
                            # multiply on the otherwise-idle gpsimd engine
                            nc.gpsimd.tensor_mul(Pt[:, c0:], Pt[:, c0:],
                                                 maskT[:, dd, c0:])
                        Pts[jc] = (Pt, c0)
                        if jc >= WIN:
                            lv(jc - WIN)
                        if fillers and (jc % fill_every == fill_every - 1):
                            fillers.pop(0)()
                    for jc in range(max(0, njc - WIN), njc):
                        lv(jc)
                    Linv = wp.tile([P, 512], F32, tag="Linv", name="Linv")
                    _recip(nc, Linv, ps_l)
                    nc.vector.tensor_mul(outT_all[:, h, isl], ps_o[:],
                                         Linv[:])
                while fillers:
                    fillers.pop(0)()
            # att(2) needs nothing from chunk 3, so it fills the PE while
            # chunk-3's ACT/DVE tail drains; the last chunk's K/V land under
            # att(2)'s matmul stream.  Out-proj of the previously finished
            # i-chunk staggers one step behind; the last two interleave into
            # att(0) so the kernel doesn't end on a long copy/DMA drain.
            emit_att(2)
            emit_K(prev)
            emit_V(prev)
            rope_flush()
            emit_att(3)
            emit_outproj(2)
            emit_att(1)
            emit_outproj(3)
            emit_att(0, fillers=outproj_units(1), fill_every=1)
            emit_outproj(0, tail=True)

    nc.compile()  # bacc passes: split multi-waits into event semaphores etc.
    _CACHE["nc"] = nc
    return nc


def _host_prep(x, Wq, bq, Wkl, bkl, t_scale, t_shift, Wc, We, ln_g, ln_b,
               Wfrom, bfrom, Wout, bout):
    """Build the 8 per-core input maps (shard + transpose + fold on host).

    Every array is laid out exactly as its SBUF tile ([P, free...]) so the
    DMA is 128 contiguous per-partition descriptors."""
    f16 = np.float16
    f32 = np.float32
    f64 = np.float64

    x2 = np.ascontiguousarray(x.reshape(T, D))
    xT = np.ascontiguousarray(x2.T).astype(f16)          # [D, T]
    # xTp[tci, p, ko*TCH + t] = xT[ko*P + p, tci*TCH + t]
    xTp = np.ascontiguousarray(
        xT.reshape(NKO, P, NT, TCH).transpose(2, 1, 0, 3).reshape(
            NT, P, NKO * TCH))

    # host-side folds (fp64 for the small chains)
    sp = np.log1p(np.exp(t_scale.astype(f64)))           # softplus
    Wfold = ((Wc.astype(f64) * sp[None, :]) @ Wkl.astype(f64))  # [DCMP, D]
    cvec = (Wc.astype(f64) @ (bkl.astype(f64) * sp + t_shift.astype(f64)))

    We_g = We.astype(f64) * ln_g.astype(f64)[None, :]
    s_vec = We_g @ np.ones(DCMP, f64)
    elp = We.astype(f64) @ ln_b.astype(f64)

    def sb_layout(wT, nko, free):
        # wT: [K, free] with K = nko*P -> [P, nko, free]
        return np.ascontiguousarray(
            wT.reshape(nko, P, free).transpose(1, 0, 2)).astype(f16)

    wfoldp = sb_layout(np.ascontiguousarray(Wfold.T.astype(f32)), NKO,
                       DCMP).reshape(P, NKO * DCMP)

    # rope tables
    t_idx = np.arange(T, dtype=f32)
    inv_freq = 1.0 / THETA ** (np.arange(0, HD, 2, dtype=f32) / HD)
    freqs = t_idx[:, None] * inv_freq[None, :]
    cosp = np.ascontiguousarray(
        np.concatenate([np.cos(freqs), np.cos(freqs)], axis=1).T).astype(f16)
    # rotation sign folded in: rot(x)[p] = -x[p+64] (p<64), +x[p-64] (p>=64)
    sinp = np.ascontiguousarray(
        np.concatenate([-np.sin(freqs), np.sin(freqs)], axis=1).T).astype(f16)

    onesp = np.ones((P, P), f16)

    # multiplicative causal masks for the 4 diagonal deltas
    # (j0 = i0 + 128*d); applied to exp(S) post-activation
    maskp = np.zeros((P, 4, 512), f16)
    pidx = np.arange(P)[:, None]
    fidx = np.arange(512)[None, :]
    for d in range(4):
        maskp[:, d, :] = np.where(pidx <= fidx - 128 * d, 1.0, 0.0)

    shared = dict(
        xTp=xTp, wfoldp=wfoldp, cosp=cosp, sinp=sinp,
        maskp=maskp, onesp=onesp,
    )

    WfK = Wfrom[:H * HD].astype(f64)
    WfV = Wfrom[H * HD:].astype(f64)
    bfK = bfrom[:H * HD].astype(f64)
    bfV = bfrom[H * HD:].astype(f64)

    in_maps = []
    for c in range(N_CORES):
        hsl = slice(c * M, (c + 1) * M)
        wqp = sb_layout(np.ascontiguousarray(Wq[hsl].T), NKO, M)
        bq_c = np.ascontiguousarray(bq[hsl].reshape(HPC, P).T).astype(f32)
        W2 = (WfK[hsl] @ We_g).astype(f32)                 # [M, DCMP]
        W3 = (WfV[hsl] @ We_g).astype(f32)
        w2p = sb_layout(np.ascontiguousarray(W2.T), NCC, M)
        w3p = sb_layout(np.ascontiguousarray(W3.T), NCC, M)
        s2 = (WfK[hsl] @ s_vec).astype(f32)
        s3 = (WfV[hsl] @ s_vec).astype(f32)
        bk_eff = (bfK[hsl] + WfK[hsl] @ elp).astype(f32)
        bv_eff = (bfV[hsl] + WfV[hsl] @ elp).astype(f32)
        bk2 = np.ascontiguousarray(bk_eff.reshape(HPC, P).T).astype(f32)
        ns2 = np.ascontiguousarray((-s2).reshape(HPC, P).T).astype(f32)
        nbvp = np.zeros((P, 2, M), f16)
        nbvp[:, 0, :] = np.broadcast_to((-s3).astype(f16)[None, :], (P, M))
        nbvp[:, 1, :] = np.broadcast_to(bv_eff.astype(f16)[None, :], (P, M))
        woutp = sb_layout(np.ascontiguousarray(Wout[:, hsl].T), HPC, D)
        vecs = np.zeros((P, 9), f32)
        vecs[:, 0:NCC] = np.ascontiguousarray(
            cvec.astype(f32).reshape(NCC, P).T)
        vecs[:, 2:2 + HPC] = bq_c
        vecs[:, 4:4 + HPC] = bk2
        vecs[:, 6:6 + HPC] = ns2
        vecs[:, 8] = LN_EPS
        in_maps.append(dict(
            shared,
            wqp=wqp, w2p=w2p, w3p=w3p, vecs=vecs,
            nbvp=nbvp, woutp=woutp,
        ))
    return in_maps


def kernel(x, Wq, bq, Wkl, bkl, t_scale, t_shift, Wc, We, ln_g, ln_b,
           Wfrom, bfrom, Wout, bout):
    global LAST_RESULT
    args = dict(x=x, Wq=Wq, bq=bq, Wkl=Wkl, bkl=bkl, t_scale=t_scale,
                t_shift=t_shift, Wc=Wc, We=We, ln_g=ln_g, ln_b=ln_b,
                Wfrom=Wfrom, bfrom=bfrom, Wout=Wout, bout=bout)
    args = {k: np.asarray(v, dtype=np.float32) for k, v in args.items()}

    nc = _build()
    in_maps = _host_prep(**args)

    want_trace = bool(int(os.environ.get("BASS_TRACE", "0") or "0"))
    try:
        res = run_bass_kernel_spmd(
            nc, in_maps, core_ids=list(range(N_CORES)), trace=want_trace,
        )
    except ModuleNotFoundError:
        os.environ["BASS_NEVER_TRACE"] = "1"
        res = run_bass_kernel_spmd(
            nc, in_maps, core_ids=list(range(N_CORES)), trace=False,
        )
    LAST_RESULT = res

    acc = np.zeros((T, D), np.float32)
    for r in res.results:
        acc += r["out_partial"].reshape(T, D).astype(np.float32)
    acc += args["bout"][None, :]
    return acc[None].astype(np.float32)


if __name__ == "__main__":
    rng = np.random.default_rng(0)
    # smoke test with random inputs (not the reference distribution)
    import reference as ref
    import jax
    with jax.default_device(jax.devices("cpu")[0]):
        inputs = {k: np.asarray(v, np.float32)
                  for k, v in ref.setup_inputs().items()}
        expected = np.asarray(ref.reference(**inputs))
    out = kernel(**inputs)
    diff = out - expected
    print("rel_fro:", np.linalg.norm(diff) / np.linalg.norm(expected))
    print("max_abs:", np.abs(diff).max())


# revision 31
# speedup vs baseline: 1.0109x; 1.0109x over previous
"""Trainium2 Bass kernel for nn_MLA_KVSplice (MLA attention with KVSplice
latent bottleneck), tensor-parallel over heads across 8 NeuronCores.

v2: the whole latent pipeline is folded on the host.  kv_latent is only
consumed by the splice, and LN is a per-token affine, so:

  y^T   = Wfold @ x^T + cvec        Wfold = Wc.diag(softplus(t_scale)).Wkl
  K^T   = (W2 @ y^T).At - s2 (mu.At)^T + bk_eff     W2 = WfK_c @ We_g
  V     = ((W3 @ y^T)^T).At_col - (mu.At) s3^T + bv_eff

so the on-device contraction chain is x[2048] -> y[256] -> K/V[256]: the
512-wide latent matmul, the compress input and the expand matmul all
disappear (~48us of PE per core vs v1).

Per core c (heads {2c, 2c+1}):
  - All big tensors live/compute in transposed layouts so every matmul
    contraction sits on the partition dim; DRAM inputs are pre-laid
    host-side in exact SBUF layout.
  - Startup: wfold + x chunk0 stream in ramped pieces on the sync+scalar
    queues; wq follows so chunk-0 Q can start ko-wise as it lands.
  - LN stats per chunk via ones-matmul (row broadcast form); column forms
    (for V) extracted from partition 0 with tiny transposing DMAs.
  - K/V of chunk i-1 are emitted inside chunk i so the PE never stalls on
    the stats DVE chain.
  - Per-head causal attention in S^T[j,i] layout: exp without
    max-subtraction, row-sums via ones-matmul, fully masked j-tiles
    skipped.
  - Row-parallel out-proj staggered into the attention loop; each core
    emits a fp16 partial [T, D]; host sums the 8 partials in fp32 + bout.
"""

import math
import os

import numpy as np

import concourse.bass as bass
import concourse.tile as tile
from concourse import bacc, mybir
from concourse.bass_utils import run_bass_kernel_spmd

# problem constants (hardcoded per harness contract)
B, T, D = 1, 2048, 2048
H, HD = 16, 128
DLAT, DCMP = 512, 256
THETA = 10000.0
LN_EPS = 1e-5
N_CORES = 8
HPC = H // N_CORES          # heads per core = 2
M = HPC * HD                # per-core head dims = 256

P = 128                     # partitions
TCH = 512                   # t-chunk for pass 1
NT = T // TCH               # 4
NKO = D // P                # 16 contraction chunks over model dim
NCC = DCMP // P             # 2
NIC = T // 512              # 4 i-chunks in attention
NJC = T // P                # 16 j-chunks
NTC16 = T // P              # 16 row-chunks in out-proj
NTS = TCH // P              # 4 t-subchunks per chunk

F16 = mybir.dt.float16
F32 = mybir.dt.float32
AF = mybir.ActivationFunctionType
ALU = mybir.AluOpType

ATT_SCALE = 1.0 / math.sqrt(HD)

_CACHE = {}

LAST_RESULT = None  # BassKernelResults of the most recent run (for test.py)


def _recip(nc, out, in_):
    """1/in_ on DVE; fast approx when available (18 bits, plenty here)."""
    if hasattr(nc.vector, "reciprocal_approx_fast"):
        nc.vector.reciprocal_approx_fast(out=out[:], in_=in_[:])
    else:
        nc.vector.reciprocal(out[:], in_[:])


def _build():
    if "nc" in _CACHE:
        return _CACHE["nc"]

    nc = bacc.Bacc(None, target_bir_lowering=False)

    def din(name, shape, dt):
        return nc.dram_tensor(name, shape, dt, kind="ExternalInput")

    # every input is pre-laid host-side in its exact SBUF layout
    xTp_d = din("xTp", [NT, P, NKO * TCH], F16)
    wfoldp_d = din("wfoldp", [P, NKO * DCMP], F16)
    wqp_d = din("wqp", [P, NKO, M], F16)
    w2p_d = din("w2p", [P, NCC, M], F16)
    w3p_d = din("w3p", [P, NCC, M], F16)
    woutp_d = din("woutp", [P, HPC, D], F16)
    cosp_d = din("cosp", [P, T], F16)
    sinp_d = din("sinp", [P, T], F16)
    maskp_d = din("maskp", [P, 4, 512], F16)
    onesp_d = din("onesp", [P, P], F16)
    # all small per-partition vectors in ONE tensor -> one DMA
    # cols: 0:2 cvec | 2:4 bq | 4:6 bk_eff | 6:8 neg_s2 | 8 eps
    vecs_d = din("vecs", [P, 9], F32)
    nbvp_d = din("nbvp", [P, 2, M], F16)   # [0]=neg_s3 bcast, [1]=bv_eff bcast

    out_d = nc.dram_tensor("out_partial", [NTC16, P, D], F16,
                           kind="ExternalOutput")

    with tile.TileContext(nc) as tc:
        with (
            tc.tile_pool(name="consts", bufs=1) as cp,
            tc.tile_pool(name="persist", bufs=1) as pp,
            tc.tile_pool(name="work", bufs=2) as wp,
            tc.tile_pool(name="psum", bufs=6, space="PSUM") as psp,
            tc.tile_pool(name="dscr", bufs=2, space="DRAM") as dp,
        ):
            # ---- tiny consts first on the gpsimd queue; the bulkier consts
            # gated behind most of the x0 stream by a blocker copy ----
            vecs = cp.tile([P, 9], F32, tag="vecs", name="vecs")
            nc.gpsimd.dma_start(vecs[:], vecs_d[:, :])
            ones16 = cp.tile([P, P], F16, tag="ones16", name="ones16")
            nc.gpsimd.dma_start(ones16[:], onesp_d[:, :])

            cvec = vecs[:, 0:NCC]
            bq2 = vecs[:, 2:2 + HPC]
            bk2 = vecs[:, 4:4 + HPC]
            ns2 = vecs[:, 6:6 + HPC]
            epsc = vecs[:, 8:9]

            # ---- all large ordered loads ride the SYNC queue alone: strict
            # arrival order, no competition for the shared DMA engines, and
            # the ACT queue stays free for copybacks.  x0+wfold stream in
            # ramped ko-pieces so the first matmul starts ~1us in. ----
            PIECES = [1, 1, 2, 4, 8]    # ko's per piece (ramped)
            wfoldT = cp.tile([P, NKO, DCMP], F16, tag="wfoldT",
                             name="wfoldT")
            xT_t0 = wp.tile([P, NKO, TCH], F16, tag="xT", name="xT_t0",
                            bufs=4)
            wf_r = wfoldp_d[:, :].rearrange("p (ko c) -> p ko c", c=DCMP)
            x0_r = xTp_d[0, :, :].rearrange("p (ko t) -> p ko t", t=TCH)
            # wq streams alongside: chunk-0 runs yT and Q interleaved
            # ko-wise, consuming each x piece once for both
            wqT = cp.tile([P, NKO, M], F16, tag="wqT", name="wqT")
            k0 = 0
            for kp in PIECES:
                ks = slice(k0, k0 + kp)
                nc.sync.dma_start(wfoldT[:, ks, :], wf_r[:, ks, :])
                nc.sync.dma_start(wqT[:, ks, :], wqp_d[:, ks, :])
                nc.scalar.dma_start(xT_t0[:, ks, :], x0_r[:, ks, :])
                k0 += kp
            xT_t1 = wp.tile([P, NKO, TCH], F16, tag="xT", name="xT_t1",
                            bufs=4)
            nc.sync.dma_start(xT_t1[:],
                              xTp_d[1, :, :].rearrange("p (ko t) -> p ko t",
                                                       t=TCH))
            cosT = cp.tile([P, T], F16, tag="cosT", name="cosT")
            nc.scalar.dma_start(cosT[:], cosp_d[:, :])
            sinT = cp.tile([P, T], F16, tag="sinT", name="sinT")
            nc.scalar.dma_start(sinT[:], sinp_d[:, :])

            # K/V weights + mask on gpsimd, behind most of the x0 stream
            blk = cp.tile([P, 1], F16, tag="blk", name="blk")
            nc.gpsimd.tensor_copy(blk[:], xT_t0[:, sum(PIECES[:3]) - 1, 0:1])
            w2T = cp.tile([P, NCC, M], F16, tag="w2T", name="w2T")
            nc.gpsimd.dma_start(w2T[:], w2p_d[:, :, :])
            w3T = cp.tile([P, NCC, M], F16, tag="w3T", name="w3T")
            nc.gpsimd.dma_start(w3T[:], w3p_d[:, :, :])
            nbv = cp.tile([P, 2, M], F16, tag="nbv", name="nbv")
            nc.gpsimd.dma_start(nbv[:], nbvp_d[:, :, :])
            maskT = cp.tile([P, 4, 512], F16, tag="maskT", name="maskT")
            nc.gpsimd.dma_start(maskT[:], maskp_d[:, :, :])
            ns3_bc = nbv[:, 0, :]
            bv_bc = nbv[:, 1, :]

            # ---- persistent intermediates ----
            kT_all = pp.tile([P, HPC, T], F16, tag="kT_all", name="kT_all")
            qT_all = pp.tile([P, HPC, T], F16, tag="qT_all", name="qT_all")
            v_all = pp.tile([P, NJC, M], F16, tag="v_all", name="v_all")
            outT_all = pp.tile([P, HPC, T], F16, tag="outT_all",
                               name="outT_all")

            # ======== pass 1: y, stats, K/V, Q (sw-pipelined) ======
            prev = {}

            HHD = HD // 2
            rope_jobs = []

            def rope_issue(pre, dst_slice, tsl):
                """Half-rotation as an SBUF->SBUF partition swap on the idle
                DMA engines (rotation sign folded into sinT host-side).  The
                DVE combine is deferred to the chunk tail."""
                perm = wp.tile([P, TCH], F16, tag="perm", name="perm", bufs=12)
                nc.gpsimd.dma_start(perm[0:HHD, :], pre[HHD:P, :])
                nc.gpsimd.dma_start(perm[HHD:P, :], pre[0:HHD, :])
                rope_jobs.append((pre, perm, dst_slice, tsl))

            def rope_flush():
                for pre, perm, dst_slice, tsl in rope_jobs:
                    t1 = wp.tile([P, TCH], F16, tag="ropet1", name="rope_t1")
                    t2 = wp.tile([P, TCH], F16, tag="ropet2", name="rope_t2")
                    nc.vector.tensor_mul(t1[:], pre[:], cosT[:, tsl])
                    nc.vector.tensor_mul(t2[:], perm[:], sinT[:, tsl])
                    nc.vector.tensor_add(dst_slice, t1[:], t2[:])
                rope_jobs.clear()

            def emit_K(pv):
                """K^T[m, t] for chunk pv: (W2@yT).At - s2.Bt^T + bk."""
                tsl = pv["tsl"]
                for h in range(HPC):
                    psk = psp.tile([P, TCH], F32, tag="ps", name="ps_k")
                    for cc in range(NCC):
                        nc.tensor.matmul(
                            psk[:],
                            w2T[:, cc, h * HD:(h + 1) * HD],
                            pv["yT"][:, cc, :],
                            start=(cc == 0), stop=(cc == NCC - 1),
                        )
                    tmp = wp.tile([P, TCH], F16, tag="ktmp", name="ktmp")
                    nc.vector.tensor_mul(tmp[:], psk[:], pv["At"][:])
                    kst = wp.tile([P, TCH], F16, tag="kst", name="kst")
                    nc.vector.scalar_tensor_tensor(
                        kst[:], pv["Bt"][:], ns2[:, h:h + 1], tmp[:],
                        op0=ALU.mult, op1=ALU.add,
                    )
                    kpre = wp.tile([P, TCH], F16, tag="kpre", name="kpre")
                    nc.scalar.activation(kpre[:], kst[:], AF.Identity,
                                         bias=bk2[:, h:h + 1])
                    rope_issue(kpre, kT_all[:, h, tsl], tsl)

            def emit_V(pv):
                """V[t, m] for chunk pv: (W3@yT)^T.At_col - Bt_col.s3 + bv."""
                tci = pv["tci"]
                for ts4 in range(NTS):
                    psv = psp.tile([P, TCH], F32, tag="ps", name="ps_v")
                    for cc in range(NCC):
                        nc.tensor.matmul(
                            psv[:, 0:M],
                            pv["yT"][:, cc, ts4 * P:(ts4 + 1) * P],
                            w3T[:, cc, :],
                            start=(cc == 0), stop=(cc == NCC - 1),
                        )
                    v1 = wp.tile([P, M], F16, tag="v1", name="v1")
                    nc.scalar.mul(v1[:], psv[:, 0:M],
                                  pv["Atc"][:, ts4:ts4 + 1])
                    v2 = wp.tile([P, M], F16, tag="v2", name="v2")
                    nc.vector.scalar_tensor_tensor(
                        v2[:], ns3_bc, pv["Btc"][:, ts4:ts4 + 1], v1[:],
                        op0=ALU.mult, op1=ALU.add,
                    )
                    nc.vector.tensor_add(v_all[:, tci * NTS + ts4, :],
                                         v2[:], bv_bc)

            def emit_Q(xT_t, tsl, psq=None):
                """Both heads; psq supplies precomputed chunk-0 psums."""
                if psq is None:
                    psq = [psp.tile([P, TCH], F32, tag="ps",
                                    name=f"ps_q{hc}")
                           for hc in range(HPC)]
                    for hc in range(HPC):
                        for ko in range(NKO):
                            nc.tensor.matmul(
                                psq[hc][:],
                                wqT[:, ko, hc * HD:(hc + 1) * HD],
                                xT_t[:, ko, :],
                                start=(ko == 0), stop=(ko == NKO - 1),
                            )
                for hc in range(HPC):
                    qpre = wp.tile([P, TCH], F16, tag="qpre", name="qpre",
                                   bufs=4)
                    nc.scalar.activation(qpre[:], psq[hc][:], AF.Identity,
                                         bias=bq2[:, hc:hc + 1])
                    rope_issue(qpre, qT_all[:, hc, tsl], tsl)

            for tci in range(NT):
                tsl = slice(tci * TCH, (tci + 1) * TCH)
                if tci == 0:
                    xT_t = xT_t0
                elif tci == 1:
                    xT_t = xT_t1
                else:
                    xT_t = wp.tile([P, NKO, TCH], F16, tag="xT", name="xT_t",
                                   bufs=4)
                    xi_r = xTp_d[tci, :, :].rearrange("p (ko t) -> p ko t",
                                                      t=TCH)
                    nc.sync.dma_start(xT_t[:], xi_r[:, :, :])
                    if tci == 2:
                        # out-proj weights (pass 2/3 only) on the scalar
                        # queue so x3 isn't delayed behind them
                        woutT = cp.tile([P, HPC, D], F16, tag="woutT",
                                        name="woutT")
                        nc.scalar.dma_start(woutT[:], woutp_d[:, :, :])

                # --- y^T = Wfold @ x^T + cvec  [DCMP, t] ---
                yT_t = wp.tile([P, NCC, TCH], F16, tag="yT", name="yT_t",
                               bufs=2)
                ysq_t = wp.tile([P, NCC, TCH], F16, tag="ysq", name="ysq_t",
                                bufs=1)
                if tci == 0:
                    # ko-outer, Q interleaved: consume each startup x piece
                    # once for both yT and Q as the pieces land
                    ps2c = [psp.tile([P, TCH], F32, tag="ps",
                                     name=f"ps_y{cc}")
                            for cc in range(NCC)]
                    psq0 = [psp.tile([P, TCH], F32, tag="ps",
                                     name=f"ps_q{hc}")
                            for hc in range(HPC)]
                    for ko in range(NKO):
                        for cc in range(NCC):
                            nc.tensor.matmul(
                                ps2c[cc][:],
                                wfoldT[:, ko, cc * P:(cc + 1) * P],
                                xT_t[:, ko, :],
                                start=(ko == 0), stop=(ko == NKO - 1),
                                skip_group_check=True,
                            )
                        for hc in range(HPC):
                            nc.tensor.matmul(
                                psq0[hc][:],
                                wqT[:, ko, hc * HD:(hc + 1) * HD],
                                xT_t[:, ko, :],
                                start=(ko == 0), stop=(ko == NKO - 1),
                                skip_group_check=True,
                            )
                    for cc in range(NCC):
                        nc.scalar.activation(yT_t[:, cc], ps2c[cc][:],
                                             AF.Identity,
                                             bias=cvec[:, cc:cc + 1])
                        nc.scalar.square(ysq_t[:, cc], yT_t[:, cc])
                else:
                    for cc in range(NCC):
                        psy = psp.tile([P, TCH], F32, tag="ps", name="ps_y")
                        for ko in range(NKO):
                            nc.tensor.matmul(
                                psy[:],
                                wfoldT[:, ko, cc * P:(cc + 1) * P],
                                xT_t[:, ko, :],
                                start=(ko == 0), stop=(ko == NKO - 1),
                            )
                        nc.scalar.activation(yT_t[:, cc], psy[:],
                                             AF.Identity,
                                             bias=cvec[:, cc:cc + 1])
                        nc.scalar.square(ysq_t[:, cc], yT_t[:, cc])

                # chunk i-1's K: fills the yT copyback latency before the
                # stats matmuls need it
                if prev:
                    emit_K(prev)

                # --- LN stats over DCMP via ones-matmul (bcast to 128p) ---
                ps1 = psp.tile([P, TCH], F32, tag="acA", name="ps_s1", bufs=1)
                ps2 = psp.tile([P, TCH], F32, tag="acB", name="ps_s2", bufs=1)
                for cc in range(NCC):
                    nc.tensor.matmul(ps1[:], ones16[:], yT_t[:, cc],
                                     start=(cc == 0), stop=(cc == NCC - 1),
                                     skip_group_check=True)
                for cc in range(NCC):
                    nc.tensor.matmul(ps2[:], ones16[:], ysq_t[:, cc],
                                     start=(cc == 0), stop=(cc == NCC - 1),
                                     skip_group_check=True)

                # chunk i-1's V (fills the stats copyback latency)
                if prev:
                    emit_V(prev)
                    if tci == NT - 1:
                        # flush K(i-1) ropes early: att(2) needs kT of
                        # chunk 2 before this chunk's Q ropes
                        rope_flush()

                mu = wp.tile([P, TCH], F32, tag="mu", name="mu", bufs=1)
                musq = wp.tile([P, TCH], F32, tag="musq", name="musq", bufs=1)
                m2 = wp.tile([P, TCH], F32, tag="m2", name="m2", bufs=1)
                std = wp.tile([P, TCH], F32, tag="std", name="std", bufs=1)
                At = wp.tile([P, TCH], F32, tag="At", name="At", bufs=2)
                Bt = wp.tile([P, TCH], F32, tag="Bt", name="Bt", bufs=2)
                nc.scalar.mul(mu[:], ps1[:], 1.0 / DCMP)
                # m2 = E[y^2] + eps  (eps folded into the copyback bias)
                nc.scalar.activation(m2[:], ps2[:], AF.Identity,
                                     bias=epsc[:, 0:1], scale=1.0 / DCMP)
                nc.scalar.square(musq[:], mu[:])
                nc.vector.tensor_sub(m2[:], m2[:], musq[:])
                nc.scalar.sqrt(std[:], m2[:])
                _recip(nc, At, std)
                nc.vector.tensor_mul(Bt[:], mu[:], At[:])
                # column forms for V: partition-0 row -> DRAM -> transposed
                # read-back (SBUF->SBUF DMAs cannot cross partitions)
                cst = dp.tile([2, TCH], F32, tag="cst", name="cst", bufs=2)
                nc.gpsimd.dma_start(cst[0:1, :], At[0:1, :])
                nc.gpsimd.dma_start(cst[1:2, :], Bt[0:1, :])
                Atc = wp.tile([P, NTS], F32, tag="Atc", name="Atc", bufs=2)
                Btc = wp.tile([P, NTS], F32, tag="Btc", name="Btc", bufs=2)
                nc.gpsimd.dma_start(
                    Atc[:], cst[0, :].rearrange("(c p) -> p c", p=P))
                nc.gpsimd.dma_start(
                    Btc[:], cst[1, :].rearrange("(c p) -> p c", p=P))

                # --- Q (+bias): independent filler over the LN chain ---
                emit_Q(xT_t, tsl, psq=(psq0 if tci == 0 else None))

                # rope combines for K(i-1) and Q(i); chunk-0's defer to
                # chunk 1 so cos/sin can load late
                if tci > 0:
                    rope_flush()

                prev = {"tci": tci, "tsl": tsl, "yT": yT_t,
                        "At": At, "Bt": Bt, "Atc": Atc, "Btc": Btc}

            # ====== pass 2 + staggered pass 3 (attention + out-proj) ======
            def outproj_unit(tt, dc, sbd, tail=False):
                """One (row-chunk, col-chunk) unit of the out projection."""
                if dc == 0:
                    sbd[tt] = wp.tile([P, D // 512, 512], F16, tag="outsb",
                                      name="out_sb", bufs=3)
                out_sb = sbd[tt]
                psp_o = psp.tile([P, 512], F32, tag="ps", name="ps_P")
                for hc in range(HPC):
                    nc.tensor.matmul(
                        psp_o[:],
                        outT_all[:, hc, tt * P:(tt + 1) * P],
                        woutT[:, hc, dc * 512:(dc + 1) * 512],
                        start=(hc == 0), stop=(hc == HPC - 1),
                    )
                # ACT is exp-loaded mid-kernel: all psum copies on DVE
                use_act = False if not tail else (dc % 2 == 0)
                if use_act:
                    nc.scalar.copy(out_sb[:, dc], psp_o[:])
                else:
                    nc.vector.tensor_copy(out_sb[:, dc], psp_o[:])
                if dc == D // 512 - 1:
                    nc.sync.dma_start(out_d[tt, :, :],
                                      out_sb[:].rearrange("p a b -> p (a b)"))

            def outproj_units(ic, tail=False):
                sbd = {}
                return [
                    (lambda tt=tt, dc=dc: outproj_unit(tt, dc, sbd, tail))
                    for tt in range(4 * ic, 4 * ic + 4)
                    for dc in range(D // 512)
                ]

            def emit_outproj(ic, tail=False):
                for u in outproj_units(ic, tail):
                    u()

            def emit_att(ic, fillers=(), fill_every=1):
                fillers = list(fillers)
                isl = slice(ic * 512, (ic + 1) * 512)
                njc = 4 * ic + 4
                for h in range(HPC):
                    ps_l = psp.tile([P, 512], F32, tag="acA", name="ps_L",
                                    bufs=1)
                    ps_o = psp.tile([P, 512], F32, tag="acB", name="ps_O",
                                    bufs=1)
                    WIN = 4
                    Pts = {}

                    def lv(jc, ps_l=ps_l, ps_o=ps_o, njc=njc, h=h, Pts=Pts):
                        Pt, c0 = Pts.pop(jc)
                        nc.tensor.matmul(ps_l[:, c0:], ones16[:], Pt[:, c0:],
                                         start=(jc == 0), stop=(jc == njc - 1),
                                         skip_group_check=True)
                        nc.tensor.matmul(ps_o[:, c0:],
                                         v_all[:, jc, h * HD:(h + 1) * HD],
                                         Pt[:, c0:],
                                         start=(jc == 0), stop=(jc == njc - 1),
                                         skip_group_check=True)

                    for jc in range(njc):
                        dd = jc - 4 * ic
                        # diagonal blocks: columns i < 128*dd are fully
                        # masked -> skip them in S/exp/L/O
                        c0 = P * dd if dd > 0 else 0
                        ps_s = psp.tile([P, 512], F32, tag="ps", name="ps_S")
                        nc.tensor.matmul(
                            ps_s[:, c0:],
                            kT_all[:, h, jc * P:(jc + 1) * P],
                            qT_all[:, h, ic * 512 + c0:(ic + 1) * 512],
                            start=True, stop=True,
                        )
                        Pt = wp.tile([P, 512], F16, tag="P", name="P_t",
                                     bufs=6)
                        nc.scalar.activation(Pt[:, c0:], ps_s[:, c0:], AF.Exp,
                                             scale=ATT_SCALE)
                        if dd >= 0:
                            # causal zeroing post-exp as a cheap fp16 SBUF
                            # multiply on the otherwise-idle gpsimd engine
                            nc.gpsimd.tensor_mul(Pt[:, c0:], Pt[:, c0:],
                                                 maskT[:, dd, c0:])
                        Pts[jc] = (Pt, c0)
                        if jc >= WIN:
                            lv(jc - WIN)
                        if fillers and (jc % fill_every == fill_every - 1):
                            fillers.pop(0)()
                    for jc in range(max(0, njc - WIN), njc):
                        lv(jc)
                    Linv = wp.tile([P, 512], F32, tag="Linv", name="Linv")
                    _recip(nc, Linv, ps_l)
                    nc.vector.tensor_mul(outT_all[:, h, isl], ps_o[:],
                                         Linv[:])
                while fillers:
                    fillers.pop(0)()
            # att(2) needs nothing from chunk 3, so it fills the PE while
            # chunk-3's ACT/DVE tail drains; the last chunk's K/V land under
            # att(2)'s matmul stream.  Out-proj of the previously finished
            # i-chunk staggers one step behind; the last two interleave into
            # att(0) so the kernel doesn't end on a long copy/DMA drain.
            emit_att(2)
            emit_K(prev)
            emit_V(prev)
            rope_flush()
            emit_att(3)
            emit_outproj(2)
            emit_att(1)
            emit_outproj(3)
            emit_att(0, fillers=outproj_units(1), fill_every=1)
            emit_outproj(0, tail=True)

    nc.compile()  # bacc passes: split multi-waits into event semaphores etc.
    _CACHE["nc"] = nc
    return nc


def _host_prep(x, Wq, bq, Wkl, bkl, t_scale, t_shift, Wc, We, ln_g, ln_b,
               Wfrom, bfrom, Wout, bout):
    """Build the 8 per-core input maps (shard + transpose + fold on host).

    Every array is laid out exactly as its SBUF tile ([P, free...]) so the
    DMA is 128 contiguous per-partition descriptors."""
    f16 = np.float16
    f32 = np.float32
    f64 = np.float64

    x2 = np.ascontiguousarray(x.reshape(T, D))
    xT = np.ascontiguousarray(x2.T).astype(f16)          # [D, T]
    # xTp[tci, p, ko*TCH + t] = xT[ko*P + p, tci*TCH + t]
    xTp = np.ascontiguousarray(
        xT.reshape(NKO, P, NT, TCH).transpose(2, 1, 0, 3).reshape(
            NT, P, NKO * TCH))

    # host-side folds (fp64 for the small chains)
    sp = np.log1p(np.exp(t_scale.astype(f64)))           # softplus
    Wfold = ((Wc.astype(f64) * sp[None, :]) @ Wkl.astype(f64))  # [DCMP, D]
    cvec = (Wc.astype(f64) @ (bkl.astype(f64) * sp + t_shift.astype(f64)))

    We_g = We.astype(f64) * ln_g.astype(f64)[None, :]
    s_vec = We_g @ np.ones(DCMP, f64)
    elp = We.astype(f64) @ ln_b.astype(f64)

    def sb_layout(wT, nko, free):
        # wT: [K, free] with K = nko*P -> [P, nko, free]
        return np.ascontiguousarray(
            wT.reshape(nko, P, free).transpose(1, 0, 2)).astype(f16)

    wfoldp = sb_layout(np.ascontiguousarray(Wfold.T.astype(f32)), NKO,
                       DCMP).reshape(P, NKO * DCMP)

    # rope tables
    t_idx = np.arange(T, dtype=f32)
    inv_freq = 1.0 / THETA ** (np.arange(0, HD, 2, dtype=f32) / HD)
    freqs = t_idx[:, None] * inv_freq[None, :]
    cosp = np.ascontiguousarray(
        np.concatenate([np.cos(freqs), np.cos(freqs)], axis=1).T).astype(f16)
    # rotation sign folded in: rot(x)[p] = -x[p+64] (p<64), +x[p-64] (p>=64)
    sinp = np.ascontiguousarray(
        np.concatenate([-np.sin(freqs), np.sin(freqs)], axis=1).T).astype(f16)

    onesp = np.ones((P, P), f16)

    # multiplicative causal masks for the 4 diagonal deltas
    # (j0 = i0 + 128*d); applied to exp(S) post-activation
    maskp = np.zeros((P, 4, 512), f16)
    pidx = np.arange(P)[:, None]
    fidx = np.arange(512)[None, :]
    for d in range(4):
        maskp[:, d, :] = np.where(pidx <= fidx - 128 * d, 1.0, 0.0)

    shared = dict(
        xTp=xTp, wfoldp=wfoldp, cosp=cosp, sinp=sinp,
        maskp=maskp, onesp=onesp,
    )

    WfK = Wfrom[:H * HD].astype(f64)
    WfV = Wfrom[H * HD:].astype(f64)
    bfK = bfrom[:H * HD].astype(f64)
    bfV = bfrom[H * HD:].astype(f64)

    in_maps = []
    for c in range(N_CORES):
        hsl = slice(c * M, (c + 1) * M)
        wqp = sb_layout(np.ascontiguousarray(Wq[hsl].T), NKO, M)
        bq_c = np.ascontiguousarray(bq[hsl].reshape(HPC, P).T).astype(f32)
        W2 = (WfK[hsl] @ We_g).astype(f32)                 # [M, DCMP]
        W3 = (WfV[hsl] @ We_g).astype(f32)
        w2p = sb_layout(np.ascontiguousarray(W2.T), NCC, M)
        w3p = sb_layout(np.ascontiguousarray(W3.T), NCC, M)
        s2 = (WfK[hsl] @ s_vec).astype(f32)
        s3 = (WfV[hsl] @ s_vec).astype(f32)
        bk_eff = (bfK[hsl] + WfK[hsl] @ elp).astype(f32)
        bv_eff = (bfV[hsl] + WfV[hsl] @ elp).astype(f32)
        bk2 = np.ascontiguousarray(bk_eff.reshape(HPC, P).T).astype(f32)
        ns2 = np.ascontiguousarray((-s2).reshape(HPC, P).T).astype(f32)
        nbvp = np.zeros((P, 2, M), f16)
        nbvp[:, 0, :] = np.broadcast_to((-s3).astype(f16)[None, :], (P, M))
        nbvp[:, 1, :] = np.broadcast_to(bv_eff.astype(f16)[None, :], (P, M))
        woutp = sb_layout(np.ascontiguousarray(Wout[:, hsl].T), HPC, D)
        vecs = np.zeros((P, 9), f32)
        vecs[:, 0:NCC] = np.ascontiguousarray(
            cvec.astype(f32).reshape(NCC, P).T)
        vecs[:, 2:2 + HPC] = bq_c
        vecs[:, 4:4 + HPC] = bk2
        vecs[:, 6:6 + HPC] = ns2
        vecs[:, 8] = LN_EPS
        in_maps.append(dict(
            shared,
            wqp=wqp, w2p=w2p, w3p=w3p, vecs=vecs,
            nbvp=nbvp, woutp=woutp,
        ))
    return in_maps


def kernel(x, Wq, bq, Wkl, bkl, t_scale, t_shift, Wc, We, ln_g, ln_b,
           Wfrom, bfrom, Wout, bout):
    global LAST_RESULT
    args = dict(x=x, Wq=Wq, bq=bq, Wkl=Wkl, bkl=bkl, t_scale=t_scale,
                t_shift=t_shift, Wc=Wc, We=We, ln_g=ln_g, ln_b=ln_b,
                Wfrom=Wfrom, bfrom=bfrom, Wout=Wout, bout=bout)
    args = {k: np.asarray(v, dtype=np.float32) for k, v in args.items()}

    nc = _build()
    in_maps = _host_prep(**args)

    want_trace = bool(int(os.environ.get("BASS_TRACE", "0") or "0"))
    try:
        res = run_bass_kernel_spmd(
            nc, in_maps, core_ids=list(range(N_CORES)), trace=want_trace,
        )
    except ModuleNotFoundError:
        os.environ["BASS_NEVER_TRACE"] = "1"
        res = run_bass_kernel_spmd(
            nc, in_maps, core_ids=list(range(N_CORES)), trace=False,
        )
    LAST_RESULT = res

    acc = np.zeros((T, D), np.float32)
    for r in res.results:
        acc += r["out_partial"].reshape(T, D).astype(np.float32)
    acc += args["bout"][None, :]
    return acc[None].astype(np.float32)


if __name__ == "__main__":
    rng = np.random.default_rng(0)
    # smoke test with random inputs (not the reference distribution)
    import reference as ref
    import jax
    with jax.default_device(jax.devices("cpu")[0]):
        inputs = {k: np.asarray(v, np.float32)
                  for k, v in ref.setup_inputs().items()}
        expected = np.asarray(ref.reference(**inputs))
    out = kernel(**inputs)
    diff = out - expected
    print("rel_fro:", np.linalg.norm(diff) / np.linalg.norm(expected))
    print("max_abs:", np.abs(diff).max())
